# revision 1
# baseline (speedup 1.0000x reference)
"""Trainium2 Bass kernel for nn_L4maAttention (llama3.1-style GQA attention layer).

Sharding: heads across 8 cores (4 Q heads + 1 KV head per core), with
on-device collectives to minimize host<->device traffic:
  - hidden_states shipped hid-sharded ([512, 2048] bf16 per core) and
    AllGathered on device in 4 pipelined chunks; projections consume
    AG blocks as they land (PSUM partials spilled to SBUF f32 between
    contraction groups).
  - q/k/v projections column-parallel + rope on device.
  - paged-KV context gathered on host, shipped pre-transposed per core
    (1 kv head per core).
  - attention per-head local in S^T layout ([kv, q]); softmax
    denominators via an f32 SBUF accumulation of the exp tiles plus a
    single all-ones matmul per head.
  - o_proj COLUMN-parallel: per-batch AllGather of the (bf16) attention
    outputs, then each core computes a disjoint 512-column slice of the
    output. No reduction; outputs return as bf16 [512, 2048] per core.

All matmul operands are bf16 (1 cycle/row on the PE).
"""

import math
import sys

import numpy as np

sys.path.insert(0, "/opt/trn_rl_repo")

import concourse.bass as bass  # noqa: E402
import concourse.mybir as mybir  # noqa: E402
import concourse.tile as tile  # noqa: E402
from concourse import bacc  # noqa: E402
from concourse.bass_utils import run_bass_kernel_spmd  # noqa: E402
from concourse.masks import make_identity  # noqa: E402

# ---- problem constants (hardcoded from spec) ----
B, QO, PAGE = 4, 512, 16
HID, HQ, HKV, D = 4096, 32, 8, 128
N = B * QO  # 2048
NCORES = 8
HQL = HQ // NCORES  # 4 local q heads
ROPE_THETA = 5e5
OLD_CTX, LOW_F, HIGH_F, RSCALE = 8192.0, 1.0, 4.0, 8.0
SM_SCALE = 1.0 / math.sqrt(D)

import ml_dtypes  # noqa: E402

BF16NP = ml_dtypes.bfloat16
FP8NP = ml_dtypes.float8_e4m3
F32 = mybir.dt.float32
BF16 = mybir.dt.bfloat16
FP8 = mybir.dt.float8e4
AF = mybir.ActivationFunctionType
ALU = mybir.AluOpType
P = 128
HS = HID // NCORES  # 512 hid rows per core for the h all-gather
JG = 4              # h all-gather split count
CW = 512            # projection moving-chunk width (tokens)
NCH = N // CW       # 4 chunks


def _llama31_inv_freq(d):
    inv = ROPE_THETA ** (-np.arange(0, d, 2, dtype=np.float32) / d)
    wavelen = 2.0 * np.pi / inv
    low_wl, high_wl = OLD_CTX / LOW_F, OLD_CTX / HIGH_F
    smooth = (OLD_CTX / wavelen - LOW_F) / (HIGH_F - LOW_F)
    mid = (1.0 - smooth) * inv / RSCALE + smooth * inv
    return np.where(
        wavelen > low_wl, inv / RSCALE, np.where(wavelen < high_wl, inv, mid)
    ).astype(np.float32)


def host_prep(inputs):
    """Shard + pre-transpose inputs for the 8 cores. Returns (in_maps, ctxl)."""
    hs = np.asarray(inputs["hidden_states"], np.float32)
    pos_ids = np.asarray(inputs["position_ids"], np.int32)
    kvc = np.asarray(inputs["kv_cache"], np.float32)
    kpi = np.asarray(inputs["kv_page_indices"], np.int32)
    kpp = np.asarray(inputs["kv_page_indptr"], np.int32)
    klp = np.asarray(inputs["kv_last_page_lens"], np.int32)
    qop = np.asarray(inputs["qo_indptr"], np.int32)
    Wq = np.asarray(inputs["Wq"], np.float32)
    Wk = np.asarray(inputs["Wk"], np.float32)
    Wv = np.asarray(inputs["Wv"], np.float32)
    Wo = np.asarray(inputs["Wo"], np.float32)

    n, hid = hs.shape
    b_sz = qop.shape[0] - 1
    qo_len = n // b_sz
    page = kvc.shape[2]
    pps = kpi.shape[0] // b_sz
    seq_len = (pps - 1) * page + klp  # [B]
    ctx_len = seq_len - qo_len
    assert n == N and hid == HID and b_sz == B and qo_len == QO
    assert np.all(ctx_len == ctx_len[0]) and int(ctx_len[0]) % 128 == 0
    ctxl = int(ctx_len[0])

    # rope tables [64, N] indexed (freq, token)
    inv = _llama31_inv_freq(D)
    ang = pos_ids.astype(np.float32)[:, None] * inv[None, :]
    cosT = np.ascontiguousarray(np.cos(ang).T).astype(BF16NP)
    sinT = np.ascontiguousarray(np.sin(ang).T).astype(BF16NP)

    # gather paged KV context (positions 0..ctxl-1 per sequence)
    cpos = np.arange(ctxl)
    pages = kpi[kpp[:-1][:, None] + (cpos[None, :] // page)]  # [B, ctxl]
    slots = np.broadcast_to(cpos % page, (b_sz, ctxl))
    Kc = kvc[pages, 0, slots]  # [B, ctxl, HKV, D]
    Vc = kvc[pages, 1, slots]

    # causal mask for the new-kv block, tiled [128, 4*512]: chunk c holds
    # rows kv_rel in [c*128,(c+1)*128) vs all 512 q_rel columns
    qr = np.arange(qo_len)
    mbig = np.where(qr[:, None] <= qr[None, :], 0.0, -1e30).astype(np.float32)
    msk = np.ascontiguousarray(
        np.concatenate([mbig[i * 128 : (i + 1) * 128] for i in range(qo_len // 128)], axis=1)
    ).astype(BF16NP)
    hT = np.ascontiguousarray(hs.T).astype(BF16NP)  # [HID, N]

    Wq4 = Wq.reshape(HQ, D, HID)
    Wk4 = Wk.reshape(HKV, D, HID)
    Wv4 = Wv.reshape(HKV, D, HID)

    in_maps = []
    for i in range(NCORES):
        hTs = np.ascontiguousarray(hT[i * HS : (i + 1) * HS, :])
        wqT = np.ascontiguousarray(Wq4[i * HQL : (i + 1) * HQL].reshape(HQL * D, HID).T).astype(BF16NP)
        wkT = np.ascontiguousarray(Wk4[i].T).astype(BF16NP)
        wvT = np.ascontiguousarray(Wv4[i].T).astype(BF16NP)
        # column-parallel o_proj slice: out cols [i*512,(i+1)*512), all head dims
        woTc = np.ascontiguousarray(Wo[i * QO : (i + 1) * QO, :].T).astype(BF16NP)  # [HQ*D, 512]
        kctxT = np.ascontiguousarray(Kc[:, :, i, :].reshape(b_sz * ctxl, D).T).astype(FP8NP)
        vctx = np.ascontiguousarray(
            Vc[:, :, i, :].reshape(-1, 128, D).transpose(1, 0, 2).reshape(128, b_sz * ctxl)
        ).astype(FP8NP)
        in_maps.append(
            dict(hTs=hTs, wqT=wqT, wkT=wkT, wvT=wvT, woTc=woTc, kctxT=kctxT,
                 vctx=vctx, cosT=cosT, sinT=sinT, msk=msk)
        )
    return in_maps, ctxl


def _rope_evict(nc, tpool, dst, src, cs, sn, w):
    """dst[0:64] = p1*cos - p2*sin ; dst[64:128] = p2*cos + p1*sin.

    src is SBUF, so each input pair must share a start partition: cs/sn
    hold the rope table duplicated in both partition halves.
    """
    t1 = tpool.tile([64, w], F32, tag="t1")
    t2 = tpool.tile([64, w], F32, tag="t2")
    t3 = tpool.tile([64, w], F32, tag="t3")
    t4 = tpool.tile([64, w], F32, tag="t4")
    nc.vector.tensor_tensor(t1[:], src[0:64, :], cs[0:64, :], ALU.mult)
    nc.vector.tensor_tensor(t2[:], src[64:128, :], sn[64:128, :], ALU.mult)
    nc.vector.tensor_tensor(dst[0:64, :], t1[:], t2[:], ALU.subtract)
    nc.gpsimd.tensor_tensor(t3[:], src[64:128, :], cs[64:128, :], ALU.mult)
    nc.gpsimd.tensor_tensor(t4[:], src[0:64, :], sn[0:64, :], ALU.mult)
    nc.gpsimd.tensor_tensor(dst[64:128, :], t3[:], t4[:], ALU.add)


def build_program(ctxl):
    KVL = ctxl + QO  # kv length per sequence
    CC = ctxl // 128  # context chunks per sequence
    KC = KVL // 128  # total kv chunks per sequence
    KH = HID // 128  # contraction chunks for projections (32)

    nc = bacc.Bacc("TRN2", debug=False, num_devices=NCORES)
    hTs = nc.dram_tensor("hTs", [HS, N], BF16, kind="ExternalInput").ap()
    wqT = nc.dram_tensor("wqT", [HID, HQL * D], BF16, kind="ExternalInput").ap()
    wkT = nc.dram_tensor("wkT", [HID, D], BF16, kind="ExternalInput").ap()
    wvT = nc.dram_tensor("wvT", [HID, D], BF16, kind="ExternalInput").ap()
    woTc = nc.dram_tensor("woTc", [HQ * D, QO], BF16, kind="ExternalInput").ap()
    kctxT = nc.dram_tensor("kctxT", [D, B * ctxl], FP8, kind="ExternalInput").ap()
    vctx = nc.dram_tensor("vctx", [P, B * ctxl], FP8, kind="ExternalInput").ap()
    cosT = nc.dram_tensor("cosT", [D // 2, N], BF16, kind="ExternalInput").ap()
    sinT = nc.dram_tensor("sinT", [D // 2, N], BF16, kind="ExternalInput").ap()
    msk = nc.dram_tensor("msk", [P, (QO // 128) * QO], BF16, kind="ExternalInput").ap()
    outT = nc.dram_tensor("outT", [QO, N], BF16, kind="ExternalOutput").ap()
    ones_c = nc.inline_tensor(np.ones((P, P), BF16NP), name="ones_c").ap()

    rg = [list(range(NCORES))]

    with tile.TileContext(nc) as tc:
        with tc.tile_pool(name="dram", bufs=1, space="DRAM") as dram:
            # ---- h all-gather, split into JG chunks along hid ----
            hins = [dram.tile([HS // JG, N], BF16, tag=f"hin{j}", name=f"hin{j}") for j in range(JG)]
            houts = [dram.tile([NCORES * (HS // JG), N], BF16, addr_space="Shared",
                               tag=f"hout{j}", name=f"hout{j}") for j in range(JG)]
            for j in range(JG):
                nc.sync.dma_start(hins[j][:], hTs[j * (HS // JG):(j + 1) * (HS // JG), :])
                nc.gpsimd.collective_compute(
                    "AllGather", ALU.bypass, replica_groups=rg,
                    ins=[hins[j][:]], outs=[houts[j][:]])
            # ---- per-batch O all-gather buffers ----
            oins = [dram.tile([HQL * D, QO], BF16, tag=f"oin{b}", name=f"oin{b}") for b in range(B)]
            oouts = [dram.tile([HQ * D, QO], BF16, addr_space="Shared",
                               tag=f"oout{b}", name=f"oout{b}") for b in range(B)]

            with tc.tile_pool(name="resident", bufs=1) as res:
                q_sb = res.tile([P, HQL * N], BF16)  # head h at cols [h*N,(h+1)*N)
                kn_sb = res.tile([P, N], BF16)  # new K^T, chunk c at cols c*CW
                vn_sb = res.tile([P, N], BF16)  # new V, 128-block t at cols t*128
                cos_sb = res.tile([P, N], BF16)
                sin_sb = res.tile([P, N], BF16)
                kctx_sb = res.tile([P, B * ctxl], BF16)
                vctx_sb = res.tile([P, B * ctxl], BF16)
                msk_sb = res.tile([P, (QO // 128) * QO], BF16)
                nc.gpsimd.dma_start(kctx_sb[:], kctxT)
                nc.gpsimd.dma_start(vctx_sb[:], vctx)
                nc.sync.dma_start(msk_sb[:], msk)
                ones_sb = res.tile([P, P], BF16)
                ident = res.tile([P, P], BF16)
                nc.scalar.dma_start(cos_sb[0:64, :], cosT)
                nc.scalar.dma_start(cos_sb[64:128, :], cosT)
                nc.scalar.dma_start(sin_sb[0:64, :], sinT)
                nc.scalar.dma_start(sin_sb[64:128, :], sinT)
                nc.scalar.dma_start(ones_sb[:], ones_c)
                make_identity(nc, ident[:])

                # ============ Phase A: QKV projections + rope ============
                with tc.tile_pool(name="wsb", bufs=1) as wpool, \
                     tc.tile_pool(name="accsb", bufs=1) as accpool, \
                     tc.tile_pool(name="hstream", bufs=12) as hpool, \
                     tc.tile_pool(name="qkvpsum", bufs=1, space="PSUM") as ppool, \
                     tc.tile_pool(name="vtpsum", bufs=2, space="PSUM") as vtpool, \
                     tc.tile_pool(name="ropetmp", bufs=1) as tpool, \
                     tc.tile_pool(name="vsb", bufs=2) as vsbpool:
                    wq_sb = wpool.tile([P, KH * HQL * D], BF16)  # (kt,m) at kt*512+m*128
                    wk_sb = wpool.tile([P, KH * D], BF16)
                    wv_sb = wpool.tile([P, KH * D], BF16)
                    for kt in range(KH):
                        nc.scalar.dma_start(wq_sb[:, kt * 512:(kt + 1) * 512],
                                            wqT[kt * 128:(kt + 1) * 128, :])
                        nc.scalar.dma_start(wk_sb[:, kt * 128:(kt + 1) * 128],
                                            wkT[kt * 128:(kt + 1) * 128, :])
                        nc.scalar.dma_start(wv_sb[:, kt * 128:(kt + 1) * 128],
                                            wvT[kt * 128:(kt + 1) * 128, :])
                    # f32 accumulators: (chunk c, m) at cols (c*6+m)*CW
                    acc = accpool.tile([P, NCH * 6 * CW], F32)
                    RPB = HS // JG // 128  # rows (128-tiles) each rank contributes per AG chunk
                    for j in range(JG):
                        for c in range(NCH):
                            ps = [ppool.tile([P, CW], F32, tag=f"m{m}", name=f"ps{m}") for m in range(6)]
                            for r in range(NCORES):
                                for jj in range(RPB):
                                    kt = r * (HS // 128) + j * RPB + jj
                                    mov = hpool.tile([P, CW], BF16)
                                    dmaq = (nc.sync, nc.scalar, nc.gpsimd)[r % 3]
                                    dmaq.dma_start(
                                        mov[:],
                                        houts[j][(r * RPB + jj) * 128:(r * RPB + jj + 1) * 128,
                                                 c * CW:(c + 1) * CW])
                                    st, sp = (r == 0 and jj == 0), (r == NCORES - 1 and jj == RPB - 1)
                                    for m in range(HQL):
                                        nc.tensor.matmul(
                                            ps[m][:],
                                            wq_sb[:, kt * 512 + m * 128: kt * 512 + (m + 1) * 128],
                                            mov[:], start=st, stop=sp)
                                    nc.tensor.matmul(
                                        ps[4][:], wk_sb[:, kt * 128:(kt + 1) * 128],
                                        mov[:], start=st, stop=sp)
                                    nc.tensor.matmul(
                                        ps[5][:], wv_sb[:, kt * 128:(kt + 1) * 128],
                                        mov[:], start=st, stop=sp)
                            for m in range(6):
                                a = acc[:, (c * 6 + m) * CW:(c * 6 + m + 1) * CW]
                                if j == 0:
                                    nc.vector.tensor_copy(a, ps[m][:])
                                else:
                                    nc.vector.tensor_tensor(a, a, ps[m][:], ALU.add)
                            if j == JG - 1:
                                cs = cos_sb[:, c * CW:(c + 1) * CW]
                                sn = sin_sb[:, c * CW:(c + 1) * CW]
                                vt = vsbpool.tile([P, CW], BF16)
                                nc.scalar.activation(
                                    vt[:], acc[:, (c * 6 + 5) * CW:(c * 6 + 5 + 1) * CW],
                                    AF.Copy)
                                for t in range(CW // 128):
                                    tp = vtpool.tile([P, P], BF16)
                                    nc.tensor.transpose(
                                        tp[:], vt[:, t * 128:(t + 1) * 128], ident[:])
                                    nc.scalar.activation(
                                        vn_sb[:, (c * (CW // 128) + t) * 128:
                                              (c * (CW // 128) + t + 1) * 128],
                                        tp[:], AF.Copy)
                                for m in range(HQL):
                                    _rope_evict(
                                        nc, tpool,
                                        q_sb[:, m * N + c * CW: m * N + (c + 1) * CW],
                                        acc[:, (c * 6 + m) * CW:(c * 6 + m + 1) * CW],
                                        cs, sn, CW)
                                _rope_evict(
                                    nc, tpool, kn_sb[:, c * CW:(c + 1) * CW],
                                    acc[:, (c * 6 + 4) * CW:(c * 6 + 4 + 1) * CW],
                                    cs, sn, CW)

                # ============ Phase B: attention (+ per-batch O AG) ============
                # ============ Phase C: column-parallel o_proj ============
                with tc.tile_pool(name="spsum", bufs=2, space="PSUM") as spool, \
                     tc.tile_pool(name="opsum", bufs=2, space="PSUM") as opool, \
                     tc.tile_pool(name="dpsum", bufs=2, space="PSUM") as dpool, \
                     tc.tile_pool(name="cpsum", bufs=2, space="PSUM") as cpool, \
                     tc.tile_pool(name="ptile", bufs=4) as p2pool, \
                     tc.tile_pool(name="rtile", bufs=2) as rpool, \
                     tc.tile_pool(name="osb", bufs=4) as osbpool, \
                     tc.tile_pool(name="wosb", bufs=1) as wopool, \
                     tc.tile_pool(name="ovstream", bufs=2) as ovpool, \
                     tc.tile_pool(name="outsb", bufs=3) as outpool:
                    woc_sb = wopool.tile([P, KH * QO], BF16)  # (kt,ob) at kt*512+ob*128
                    for kt in range(KH):
                        nc.sync.dma_start(woc_sb[:, kt * 512:(kt + 1) * 512],
                                          woTc[kt * 128:(kt + 1) * 128, :])

                    def attn_batch(b):
                        for h in range(HQL):
                            po = opool.tile([P, QO], F32)
                            pd = dpool.tile([P, QO], F32)
                            qap = q_sb[:, h * N + b * QO: h * N + (b + 1) * QO]

                            def vslice(ckv):
                                if ckv < CC:
                                    return vctx_sb[:, b * ctxl + ckv * 128: b * ctxl + (ckv + 1) * 128]
                                jn = ckv - CC
                                return vn_sb[:, (b * 4 + jn) * 128: (b * 4 + jn + 1) * 128]

                            pts = [None] * KC
                            for ckv in range(KC):
                                if ckv < CC:
                                    kl = kctx_sb[:, b * ctxl + ckv * 128: b * ctxl + (ckv + 1) * 128]
                                else:
                                    jn = ckv - CC
                                    kl = kn_sb[:, b * QO + jn * 128: b * QO + (jn + 1) * 128]
                                st = spool.tile([P, QO], F32)
                                nc.tensor.matmul(st[:], kl, qap, start=True, stop=True)
                                if ckv >= CC:
                                    jn = ckv - CC
                                    nc.vector.tensor_tensor(
                                        st[:], st[:], msk_sb[:, jn * QO:(jn + 1) * QO],
                                        ALU.add)
                                pt = p2pool.tile([P, QO], BF16)
                                nc.scalar.activation(pt[:], st[:], AF.Exp, scale=SM_SCALE)
                                pts[ckv] = pt
                                if ckv > 0:
                                    nc.tensor.matmul(po[:], vslice(ckv - 1), pts[ckv - 1][:],
                                                     start=(ckv - 1 == 0), stop=False)
                                    nc.tensor.matmul(pd[:], ones_sb[:], pts[ckv - 1][:],
                                                     start=(ckv - 1 == 0), stop=False)
                            nc.tensor.matmul(po[:], vslice(KC - 1), pts[KC - 1][:],
                                             start=False, stop=True)
                            nc.tensor.matmul(pd[:], ones_sb[:], pts[KC - 1][:],
                                             start=False, stop=True)
                            rsb = rpool.tile([P, QO], F32, tag="rsb")
                            nc.vector.reciprocal(rsb[:], pd[:])
                            ot = osbpool.tile([P, QO], BF16)
                            nc.vector.tensor_tensor(ot[:], po[:], rsb[:], ALU.mult)
                            nc.sync.dma_start(
                                oins[b][h * 128:(h + 1) * 128, :], ot[:])
                        nc.gpsimd.collective_compute(
                            "AllGather", ALU.bypass, replica_groups=rg,
                            ins=[oins[b][:]], outs=[oouts[b][:]])

                    def prefetch_ovs(b):
                        ovs = []
                        for kt in range(KH):
                            ov = ovpool.tile([P, QO], BF16, name=f"ov{kt}")
                            nc.sync.dma_start(ov[:], oouts[b][kt * 128:(kt + 1) * 128, :])
                            ovs.append(ov)
                        return ovs

                    def oproj_batch(b, ovs):
                        for ob in range(QO // 128):
                            pc = cpool.tile([P, QO], F32)
                            for kt in range(KH):
                                nc.tensor.matmul(
                                    pc[:], woc_sb[:, kt * 512 + ob * 128: kt * 512 + (ob + 1) * 128],
                                    ovs[kt][:], start=(kt == 0), stop=(kt == KH - 1))
                            ot2 = outpool.tile([P, QO], BF16)
                            nc.scalar.activation(ot2[:], pc[:], AF.Copy)
                            nc.sync.dma_start(
                                outT[ob * 128:(ob + 1) * 128, b * QO:(b + 1) * QO], ot2[:])

                    pend = None
                    for b in range(B):
                        attn_batch(b)
                        ovs_b = prefetch_ovs(b)
                        if pend is not None:
                            oproj_batch(b - 1, pend)
                        pend = ovs_b
                    oproj_batch(B - 1, pend)
    nc.compile()
    return nc


_NC_CACHE = {}


def _get_program(ctxl):
    if ctxl not in _NC_CACHE:
        _NC_CACHE[ctxl] = build_program(ctxl)
    return _NC_CACHE[ctxl]


def run(inputs, trace=False):
    in_maps, ctxl = host_prep(inputs)
    nc = _get_program(ctxl)
    kw = dict(tmpdir="/tmp/trace_out") if trace else {}
    res = run_bass_kernel_spmd(nc, in_maps, core_ids=list(range(NCORES)), trace=trace, **kw)
    out = np.empty((N, HID), np.float32)
    for i, r in enumerate(res.results):
        out[:, i * QO:(i + 1) * QO] = np.asarray(r["outT"]).T.astype(np.float32)
    return out, res


def kernel(**inputs) -> np.ndarray:
    out, _ = run(inputs, trace=False)
    return out



# revision 6
# speedup vs baseline: 1.0926x; 1.0926x over previous
"""Trainium2 Bass kernel for nn_L4maAttention (llama3.1-style GQA attention layer).

Sharding: heads across 8 cores (4 Q heads + 1 KV head per core).
  - full hidden_states shipped (bf16, transposed) to every core from host;
    no on-device h AllGather -> projections start immediately.
  - q/k/v projections column-parallel + rope on device. Single-pass PSUM
    accumulation over the full 4096 contraction (quarter-token accumulator
    tiles), evicted via scalar copy + rope on vector/gpsimd.
  - paged-KV context gathered on host, shipped pre-transposed per core
    (1 kv head per core, fp8 in DRAM, bf16 in SBUF via dge-cast).
  - attention per-head local in S^T layout ([kv, q]); 2-head groups
    (1024-wide tiles); softmax denominators via ones-matmul accumulated in
    PSUM; reciprocal_approx_fast; masks added on DVE for the 4 new-kv chunks.
  - o_proj COLUMN-parallel: per-batch AllGather of bf16 attention outputs,
    each core computes a disjoint 512-column slice. Output bf16 [512, 2048].

All matmul operands bf16 (fp8 beyond the kv-ctx DRAM encoding fails the
2e-2 accuracy budget; measured on host).
"""

import math
import sys

import numpy as np

sys.path.insert(0, "/opt/trn_rl_repo")

import concourse.bass as bass  # noqa: E402
import concourse.mybir as mybir  # noqa: E402
import concourse.tile as tile  # noqa: E402
from concourse import bacc  # noqa: E402
from concourse.bass_utils import run_bass_kernel_spmd  # noqa: E402
from concourse.masks import make_identity  # noqa: E402

import ml_dtypes  # noqa: E402

# ---- problem constants (hardcoded from spec) ----
B, QO, PAGE = 4, 512, 16
HID, HQ, HKV, D = 4096, 32, 8, 128
N = B * QO  # 2048
NCORES = 8
HQL = HQ // NCORES  # 4 local q heads
ROPE_THETA = 5e5
OLD_CTX, LOW_F, HIGH_F, RSCALE = 8192.0, 1.0, 4.0, 8.0
SM_SCALE = 1.0 / math.sqrt(D)

BF16NP = ml_dtypes.bfloat16
FP8NP = ml_dtypes.float8_e4m3
F32 = mybir.dt.float32
BF16 = mybir.dt.bfloat16
FP8 = mybir.dt.float8e4
AF = mybir.ActivationFunctionType
ALU = mybir.AluOpType
P = 128
KH = HID // P  # 32 contraction chunks


def _llama31_inv_freq(d):
    inv = ROPE_THETA ** (-np.arange(0, d, 2, dtype=np.float32) / d)
    wavelen = 2.0 * np.pi / inv
    low_wl, high_wl = OLD_CTX / LOW_F, OLD_CTX / HIGH_F
    smooth = (OLD_CTX / wavelen - LOW_F) / (HIGH_F - LOW_F)
    mid = (1.0 - smooth) * inv / RSCALE + smooth * inv
    return np.where(
        wavelen > low_wl, inv / RSCALE, np.where(wavelen < high_wl, inv, mid)
    ).astype(np.float32)


def host_prep(inputs):
    """Shard + pre-transpose inputs for the 8 cores. Returns (in_maps, ctxl)."""
    hs = np.asarray(inputs["hidden_states"], np.float32)
    pos_ids = np.asarray(inputs["position_ids"], np.int32)
    kvc = np.asarray(inputs["kv_cache"], np.float32)
    kpi = np.asarray(inputs["kv_page_indices"], np.int32)
    kpp = np.asarray(inputs["kv_page_indptr"], np.int32)
    klp = np.asarray(inputs["kv_last_page_lens"], np.int32)
    qop = np.asarray(inputs["qo_indptr"], np.int32)
    Wq = np.asarray(inputs["Wq"], np.float32)
    Wk = np.asarray(inputs["Wk"], np.float32)
    Wv = np.asarray(inputs["Wv"], np.float32)
    Wo = np.asarray(inputs["Wo"], np.float32)

    n, hid = hs.shape
    b_sz = qop.shape[0] - 1
    qo_len = n // b_sz
    page = kvc.shape[2]
    pps = kpi.shape[0] // b_sz
    seq_len = (pps - 1) * page + klp  # [B]
    ctx_len = seq_len - qo_len
    assert n == N and hid == HID and b_sz == B and qo_len == QO
    assert np.all(ctx_len == ctx_len[0]) and int(ctx_len[0]) % 128 == 0
    ctxl = int(ctx_len[0])

    # rope tables [64, N] indexed (freq, token)
    inv = _llama31_inv_freq(D)
    ang = pos_ids.astype(np.float32)[:, None] * inv[None, :]
    cosT = np.ascontiguousarray(np.cos(ang).T).astype(BF16NP)
    sinT = np.ascontiguousarray(np.sin(ang).T).astype(BF16NP)

    # gather paged KV context (positions 0..ctxl-1 per sequence)
    cpos = np.arange(ctxl)
    pages = kpi[kpp[:-1][:, None] + (cpos[None, :] // page)]  # [B, ctxl]
    slots = np.broadcast_to(cpos % page, (b_sz, ctxl))
    Kc = kvc[pages, 0, slots]  # [B, ctxl, HKV, D]
    Vc = kvc[pages, 1, slots]

    # per-chunk causal mask for the new-kv block, [128, 4*512]:
    # chunk jn holds rows kv_rel in [jn*128,(jn+1)*128) vs all 512 q_rel cols
    qr = np.arange(qo_len)
    mbig = np.where(qr[:, None] <= qr[None, :], 0.0, -1e30).astype(np.float32)
    msk = np.ascontiguousarray(
        np.concatenate(
            [mbig[i * 128 : (i + 1) * 128] for i in range(qo_len // 128)], axis=1
        )
    ).astype(BF16NP)
    hT = np.ascontiguousarray(hs.T).astype(BF16NP)  # [HID, N] (full, all cores)

    Wq4 = Wq.reshape(HQ, D, HID)
    Wk4 = Wk.reshape(HKV, D, HID)
    Wv4 = Wv.reshape(HKV, D, HID)

    in_maps = []
    for i in range(NCORES):
        wqT = np.ascontiguousarray(
            Wq4[i * HQL : (i + 1) * HQL].reshape(HQL * D, HID).T
        ).astype(BF16NP)
        wkT = np.ascontiguousarray(Wk4[i].T).astype(BF16NP)
        wvT = np.ascontiguousarray(Wv4[i].T).astype(BF16NP)
        # column-parallel o_proj slice: out cols [i*512,(i+1)*512)
        woTc = np.ascontiguousarray(Wo[i * QO : (i + 1) * QO, :].T).astype(BF16NP)
        kctxT = np.ascontiguousarray(
            Kc[:, :, i, :].reshape(b_sz * ctxl, D).T
        ).astype(FP8NP)
        vctx = np.ascontiguousarray(
            Vc[:, :, i, :].reshape(-1, 128, D).transpose(1, 0, 2).reshape(128, b_sz * ctxl)
        ).astype(FP8NP)
        in_maps.append(
            dict(hT=hT, wqT=wqT, wkT=wkT, wvT=wvT, woTc=woTc, kctxT=kctxT,
                 vctx=vctx, cosT=cosT, sinT=sinT, msk=msk)
        )
    return in_maps, ctxl


def build_program(ctxl):
    KVL = ctxl + QO  # kv length per sequence (2048)
    CC = ctxl // 128  # context chunks per sequence (12)
    KC = KVL // 128  # total kv chunks per sequence (16)

    nc = bacc.Bacc("TRN2", debug=False, num_devices=NCORES)
    hT = nc.dram_tensor("hT", [HID, N], BF16, kind="ExternalInput").ap()
    wqT = nc.dram_tensor("wqT", [HID, HQL * D], BF16, kind="ExternalInput").ap()
    wkT = nc.dram_tensor("wkT", [HID, D], BF16, kind="ExternalInput").ap()
    wvT = nc.dram_tensor("wvT", [HID, D], BF16, kind="ExternalInput").ap()
    woTc = nc.dram_tensor("woTc", [HQ * D, QO], BF16, kind="ExternalInput").ap()
    kctxT = nc.dram_tensor("kctxT", [D, B * ctxl], FP8, kind="ExternalInput").ap()
    vctx = nc.dram_tensor("vctx", [P, B * ctxl], FP8, kind="ExternalInput").ap()
    cosT = nc.dram_tensor("cosT", [D // 2, N], BF16, kind="ExternalInput").ap()
    sinT = nc.dram_tensor("sinT", [D // 2, N], BF16, kind="ExternalInput").ap()
    msk = nc.dram_tensor("msk", [P, (QO // 128) * QO], BF16, kind="ExternalInput").ap()
    outT = nc.dram_tensor("outT", [QO, N], BF16, kind="ExternalOutput").ap()
    ones_c = nc.inline_tensor(np.ones((P, P), BF16NP), name="ones_c").ap()

    rg = [list(range(NCORES))]

    with tile.TileContext(nc) as tc:
        with tc.tile_pool(name="dram", bufs=1, space="DRAM") as dram:
            # tiny warm-up AllGather to absorb the first-collective barrier
            # while projections run
            wu_in = dram.tile([P, 8], BF16, name="wu_in")
            wu_out = dram.tile([NCORES * P, 8], BF16, addr_space="Shared", name="wu_out")
            nc.gpsimd.collective_compute(
                "AllGather", ALU.bypass, replica_groups=rg,
                ins=[wu_in[:]], outs=[wu_out[:]])
            # per-batch O all-gather buffers
            oins = [dram.tile([HQL * D, QO], BF16, tag=f"oin{b}", name=f"oin{b}")
                    for b in range(B)]
            oouts = [dram.tile([HQ * D, QO], BF16, addr_space="Shared",
                               tag=f"oout{b}", name=f"oout{b}") for b in range(B)]

            with tc.tile_pool(name="resident", bufs=1) as res:
                # q_sb: batch-major: col = b*2048 + m*512 + q
                q_sb = res.tile([P, HQL * N], BF16)
                kn_sb = res.tile([P, N], BF16)   # new K^T: [d, global token]
                vn_sb = res.tile([P, N], BF16)   # new V: 128-block t at cols t*128
                cos_sb = res.tile([P, N], BF16)
                sin_sb = res.tile([P, N], BF16)
                kctx_sb = res.tile([P, B * ctxl], BF16)
                vctx_sb = res.tile([P, B * ctxl], BF16)
                msk_sb = res.tile([P, (QO // 128) * QO], BF16)
                ones_sb = res.tile([P, P], BF16)
                ident = res.tile([P, P], BF16)
                nc.gpsimd.dma_start(kctx_sb[:], kctxT)
                nc.gpsimd.dma_start(vctx_sb[:], vctx)
                nc.sync.dma_start(msk_sb[:], msk)
                nc.scalar.dma_start(cos_sb[0:64, :], cosT)
                nc.scalar.dma_start(cos_sb[64:128, :], cosT)
                nc.scalar.dma_start(sin_sb[0:64, :], sinT)
                nc.scalar.dma_start(sin_sb[64:128, :], sinT)
                nc.scalar.dma_start(ones_sb[:], ones_c)
                make_identity(nc, ident[:])

                # ============ Phase A: QKV projections + rope ============
                with tc.tile_pool(name="wsb", bufs=1) as wpool, \
                     tc.tile_pool(name="hstream", bufs=36) as hpool, \
                     tc.tile_pool(name="evtmp", bufs=2) as epool, \
                     tc.tile_pool(name="apsum", bufs=5, space="PSUM") as apool, \
                     tc.tile_pool(name="tpsum", bufs=2, space="PSUM") as tpool:
                    wq_sb = wpool.tile([P, KH * HQL * D], BF16)  # (kt,m) at kt*512+m*128
                    wk_sb = wpool.tile([P, KH * D], BF16)
                    wv_sb = wpool.tile([P, KH * D], BF16)
                    for kt in range(KH):
                        dq = (nc.sync, nc.scalar, nc.gpsimd)[kt % 3]
                        dq.dma_start(wq_sb[:, kt * 512:(kt + 1) * 512],
                                     wqT[kt * 128:(kt + 1) * 128, :])
                        dq.dma_start(wk_sb[:, kt * 128:(kt + 1) * 128],
                                     wkT[kt * 128:(kt + 1) * 128, :])
                        dq.dma_start(wv_sb[:, kt * 128:(kt + 1) * 128],
                                     wvT[kt * 128:(kt + 1) * 128, :])

                    def wslice(m, kt):
                        if m < HQL:
                            return wq_sb[:, kt * 512 + m * 128: kt * 512 + (m + 1) * 128]
                        if m == HQL:
                            return wk_sb[:, kt * 128:(kt + 1) * 128]
                        return wv_sb[:, kt * 128:(kt + 1) * 128]

                    def rope_evict(dst_ap, src_sb, qoff):
                        """rope from f32 SBUF tile [128,512] -> dst bf16 [128,512]."""
                        cs = cos_sb[:, qoff:qoff + 512]
                        sn = sin_sb[:, qoff:qoff + 512]
                        t1 = epool.tile([64, 512], F32, tag="t1")
                        t2 = epool.tile([64, 512], F32, tag="t2")
                        t3 = epool.tile([64, 512], F32, tag="t3")
                        t4 = epool.tile([64, 512], F32, tag="t4")
                        nc.vector.tensor_tensor(t1[:], src_sb[0:64, :], cs[0:64, :], ALU.mult)
                        nc.vector.tensor_tensor(t2[:], src_sb[64:128, :], sn[64:128, :], ALU.mult)
                        nc.vector.tensor_tensor(dst_ap[0:64, :], t1[:], t2[:], ALU.subtract)
                        nc.gpsimd.tensor_tensor(t3[:], src_sb[64:128, :], cs[64:128, :], ALU.mult)
                        nc.gpsimd.tensor_tensor(t4[:], src_sb[0:64, :], sn[0:64, :], ALU.mult)
                        nc.gpsimd.tensor_tensor(dst_ap[64:128, :], t3[:], t4[:], ALU.add)

                    # h tiles per (half, kt): [128, 1024]
                    hts = {}
                    for half in range(2):
                        for kt in range(KH):
                            t = hpool.tile([P, 1024], BF16, tag="h", name=f"h{half}_{kt}")
                            dq = (nc.sync, nc.scalar, nc.gpsimd)[kt % 3]
                            dq.dma_start(
                                t[:], hT[kt * 128:(kt + 1) * 128,
                                         half * 1024:(half + 1) * 1024])
                            hts[(half, kt)] = t
                        for qq in range(2):
                            quarter = half * 2 + qq
                            qoff = quarter * 512
                            b = quarter  # quarter == batch index
                            for m in range(6):
                                acc = apool.tile([P, 512], F32, tag="acc", name=f"acc{quarter}_{m}")
                                for kt in range(KH):
                                    nc.tensor.matmul(
                                        acc[:], wslice(m, kt),
                                        hts[(half, kt)][:, qq * 512:(qq + 1) * 512],
                                        start=(kt == 0), stop=(kt == KH - 1))
                                if m < 5:
                                    asb = epool.tile([P, 512], F32, tag="asb")
                                    nc.scalar.activation(asb[:], acc[:], AF.Copy)
                                    if m < HQL:
                                        dst = q_sb[:, b * (HQL * QO) + m * QO:
                                                   b * (HQL * QO) + (m + 1) * QO]
                                    else:
                                        dst = kn_sb[:, qoff:qoff + 512]
                                    rope_evict(dst, asb, qoff)
                                else:
                                    vt = epool.tile([P, 512], BF16, tag="vt")
                                    nc.scalar.activation(vt[:], acc[:], AF.Copy)
                                    tp = tpool.tile([P, 4, P], BF16, tag="tp", name=f"tp{quarter}")
                                    for t in range(4):
                                        nc.tensor.transpose(
                                            tp[:, t, :], vt[:, t * 128:(t + 1) * 128],
                                            ident[:])
                                    nc.vector.tensor_copy(
                                        vn_sb[:, qoff:qoff + 512], tp[:, :, :])

                # ============ Phase B: attention ============
                with tc.tile_pool(name="wosb", bufs=1) as wopool, \
                     tc.tile_pool(name="ovstream", bufs=32) as ovpool:
                    woc_sb = wopool.tile([P, KH * QO], BF16)  # (kt,ob) at kt*512+ob*128
                    for kt in range(KH):
                        nc.sync.dma_start(woc_sb[:, kt * 512:(kt + 1) * 512],
                                          woTc[kt * 128:(kt + 1) * 128, :])
                    ovs = {}

                    def prefetch_ovs(bp):
                        for kt in range(KH):
                            ov = ovpool.tile([P, 1024], BF16, tag="ov", name=f"ov{bp}_{kt}")
                            dq = (nc.sync, nc.scalar)[kt % 2]
                            dq.dma_start(ov[:, 0:512],
                                         oouts[2 * bp][kt * 128:(kt + 1) * 128, :])
                            dq.dma_start(ov[:, 512:1024],
                                         oouts[2 * bp + 1][kt * 128:(kt + 1) * 128, :])
                            ovs[(bp, kt)] = ov

                    with tc.tile_pool(name="spsum", bufs=2, space="PSUM") as spool, \
                         tc.tile_pool(name="opsum", bufs=1, space="PSUM") as opool, \
                         tc.tile_pool(name="dpsum", bufs=1, space="PSUM") as dpool, \
                         tc.tile_pool(name="ptile", bufs=4) as ppool, \
                         tc.tile_pool(name="rtile", bufs=2) as rpool, \
                         tc.tile_pool(name="osb", bufs=4) as osbpool:
                        for b in range(B):
                            for hg in range(2):
                                po = opool.tile([P, 1024], F32, tag="po", name=f"po{b}_{hg}")
                                pd = dpool.tile([P, 1024], F32, tag="pd", name=f"pd{b}_{hg}")
                                qbase = b * (HQL * QO) + hg * 1024
                                for c in range(KC):
                                    if c < CC:
                                        kl = kctx_sb[:, b * ctxl + c * 128:
                                                     b * ctxl + (c + 1) * 128]
                                    else:
                                        jn = c - CC
                                        kl = kn_sb[:, b * QO + jn * 128:
                                                   b * QO + (jn + 1) * 128]
                                    st = spool.tile([P, 1024], F32, tag="st", name=f"st{c%2}")
                                    for i in range(2):
                                        nc.tensor.matmul(
                                            st[:, i * 512:(i + 1) * 512], kl,
                                            q_sb[:, qbase + i * 512: qbase + (i + 1) * 512],
                                            start=True, stop=True)
                                    if c >= CC:
                                        jn = c - CC
                                        for i in range(2):
                                            nc.vector.tensor_tensor(
                                                st[:, i * 512:(i + 1) * 512],
                                                st[:, i * 512:(i + 1) * 512],
                                                msk_sb[:, jn * QO:(jn + 1) * QO],
                                                ALU.add)
                                    pt = ppool.tile([P, 1024], BF16, tag="pt", name=f"pt{c%4}")
                                    nc.scalar.activation(pt[:], st[:], AF.Exp,
                                                         scale=SM_SCALE)
                                    if c < CC:
                                        vsl = vctx_sb[:, (b * CC + c) * 128:
                                                      (b * CC + c + 1) * 128]
                                    else:
                                        jn = c - CC
                                        vsl = vn_sb[:, (b * 4 + jn) * 128:
                                                    (b * 4 + jn + 1) * 128]
                                    st_, sp_ = (c == 0), (c == KC - 1)
                                    for i in range(2):
                                        nc.tensor.matmul(
                                            po[:, i * 512:(i + 1) * 512], vsl,
                                            pt[:, i * 512:(i + 1) * 512],
                                            start=st_, stop=sp_)
                                    for i in range(2):
                                        nc.tensor.matmul(
                                            pd[:, i * 512:(i + 1) * 512], ones_sb[:],
                                            pt[:, i * 512:(i + 1) * 512],
                                            start=st_, stop=sp_)
                                rsb = rpool.tile([P, 1024], F32, tag="rsb")
                                nc.vector.reciprocal_approx_fast(rsb[:], pd[:])
                                ot = osbpool.tile([P, 1024], BF16)
                                nc.vector.tensor_tensor(ot[:], po[:], rsb[:], ALU.mult)
                                for i in range(2):
                                    m = hg * 2 + i
                                    nc.sync.dma_start(
                                        oins[b][m * 128:(m + 1) * 128, :],
                                        ot[:, i * 512:(i + 1) * 512])
                            nc.gpsimd.collective_compute(
                                "AllGather", ALU.bypass, replica_groups=rg,
                                ins=[oins[b][:]], outs=[oouts[b][:]])
                            # prefetch AG results for o_proj as soon as ready
                            if b == 1:
                                prefetch_ovs(0)

                    # ============ Phase C: column-parallel o_proj ============
                    with tc.tile_pool(name="cpsum", bufs=4, space="PSUM") as cpool, \
                         tc.tile_pool(name="outsb", bufs=3) as outpool:
                        for bp in range(2):
                            pcs = [cpool.tile([P, 1024], F32, tag="pc", name=f"pc{bp}_{ob}")
                                   for ob in range(QO // 128)]
                            for kt in range(KH):
                                for ob in range(QO // 128):
                                    for i in range(2):
                                        nc.tensor.matmul(
                                            pcs[ob][:, i * 512:(i + 1) * 512],
                                            woc_sb[:, kt * 512 + ob * 128:
                                                   kt * 512 + (ob + 1) * 128],
                                            ovs[(bp, kt)][:, i * 512:(i + 1) * 512],
                                            start=(kt == 0), stop=(kt == KH - 1))
                            if bp == 0:
                                prefetch_ovs(1)
                            for ob in range(QO // 128):
                                ot2 = outpool.tile([P, 1024], BF16)
                                nc.vector.tensor_copy(ot2[:], pcs[ob][:])
                                for i in range(2):
                                    bb = 2 * bp + i
                                    nc.sync.dma_start(
                                        outT[ob * 128:(ob + 1) * 128,
                                             bb * QO:(bb + 1) * QO],
                                        ot2[:, i * 512:(i + 1) * 512])
    nc.compile()
    return nc


_NC_CACHE = {}


def _get_program(ctxl):
    if ctxl not in _NC_CACHE:
        _NC_CACHE[ctxl] = build_program(ctxl)
    return _NC_CACHE[ctxl]


def run(inputs, trace=False):
    in_maps, ctxl = host_prep(inputs)
    nc = _get_program(ctxl)
    kw = dict(tmpdir="/tmp/trace_out") if trace else {}
    res = run_bass_kernel_spmd(nc, in_maps, core_ids=list(range(NCORES)), trace=trace, **kw)
    out = np.empty((N, HID), np.float32)
    for i, r in enumerate(res.results):
        out[:, i * QO:(i + 1) * QO] = np.asarray(r["outT"]).T.astype(np.float32)
    return out, res


def kernel(**inputs) -> np.ndarray:
    out, _ = run(inputs, trace=False)
    return out


# revision 14
# speedup vs baseline: 1.3323x; 1.2194x over previous
"""Trainium2 Bass kernel for nn_L4maAttention (llama3.1-style GQA attention layer).

Sharding: heads across 8 cores (4 Q heads + 1 KV head per core).
  - full hidden_states shipped (bf16, transposed) to every core from host;
    no on-device h AllGather -> projections start immediately.
  - q/k/v projections column-parallel + rope on device. Single-pass PSUM
    accumulation over the full 4096 contraction (quarter-token accumulator
    tiles), evicted via scalar copy + rope on vector/gpsimd.
  - paged-KV context gathered on host, shipped pre-transposed per core
    (1 kv head per core, fp8 in DRAM, bf16 in SBUF via dge-cast).
  - attention per-head local in S^T layout ([kv, q]); 2-head groups
    (1024-wide tiles); softmax denominators via ones-matmul accumulated in
    PSUM; reciprocal_approx_fast; masks added on DVE for the 4 new-kv chunks.
  - o_proj COLUMN-parallel: per-batch AllGather of bf16 attention outputs,
    each core computes a disjoint 512-column slice. Output bf16 [512, 2048].

All matmul operands bf16 (fp8 beyond the kv-ctx DRAM encoding fails the
2e-2 accuracy budget; measured on host).
"""

import math
import sys

import numpy as np

sys.path.insert(0, "/opt/trn_rl_repo")

import concourse.bass as bass  # noqa: E402
import concourse.mybir as mybir  # noqa: E402
import concourse.tile as tile  # noqa: E402
from concourse import bacc  # noqa: E402
from concourse.bass_utils import run_bass_kernel_spmd  # noqa: E402
from concourse.masks import make_identity  # noqa: E402

import ml_dtypes  # noqa: E402

# ---- problem constants (hardcoded from spec) ----
B, QO, PAGE = 4, 512, 16
HID, HQ, HKV, D = 4096, 32, 8, 128
N = B * QO  # 2048
NCORES = 8
HQL = HQ // NCORES  # 4 local q heads
ROPE_THETA = 5e5
OLD_CTX, LOW_F, HIGH_F, RSCALE = 8192.0, 1.0, 4.0, 8.0
SM_SCALE = 1.0 / math.sqrt(D)

BF16NP = ml_dtypes.bfloat16
FP8NP = ml_dtypes.float8_e4m3
F32 = mybir.dt.float32
BF16 = mybir.dt.bfloat16
FP8 = mybir.dt.float8e4
AF = mybir.ActivationFunctionType
ALU = mybir.AluOpType
P = 128
KH = HID // P  # 32 contraction chunks


def _llama31_inv_freq(d):
    inv = ROPE_THETA ** (-np.arange(0, d, 2, dtype=np.float32) / d)
    wavelen = 2.0 * np.pi / inv
    low_wl, high_wl = OLD_CTX / LOW_F, OLD_CTX / HIGH_F
    smooth = (OLD_CTX / wavelen - LOW_F) / (HIGH_F - LOW_F)
    mid = (1.0 - smooth) * inv / RSCALE + smooth * inv
    return np.where(
        wavelen > low_wl, inv / RSCALE, np.where(wavelen < high_wl, inv, mid)
    ).astype(np.float32)


def host_prep(inputs):
    """Shard + pre-transpose inputs for the 8 cores. Returns (in_maps, ctxl)."""
    hs = np.asarray(inputs["hidden_states"], np.float32)
    pos_ids = np.asarray(inputs["position_ids"], np.int32)
    kvc = np.asarray(inputs["kv_cache"], np.float32)
    kpi = np.asarray(inputs["kv_page_indices"], np.int32)
    kpp = np.asarray(inputs["kv_page_indptr"], np.int32)
    klp = np.asarray(inputs["kv_last_page_lens"], np.int32)
    qop = np.asarray(inputs["qo_indptr"], np.int32)
    Wq = np.asarray(inputs["Wq"], np.float32)
    Wk = np.asarray(inputs["Wk"], np.float32)
    Wv = np.asarray(inputs["Wv"], np.float32)
    Wo = np.asarray(inputs["Wo"], np.float32)

    n, hid = hs.shape
    b_sz = qop.shape[0] - 1
    qo_len = n // b_sz
    page = kvc.shape[2]
    pps = kpi.shape[0] // b_sz
    seq_len = (pps - 1) * page + klp  # [B]
    ctx_len = seq_len - qo_len
    assert n == N and hid == HID and b_sz == B and qo_len == QO
    assert np.all(ctx_len == ctx_len[0]) and int(ctx_len[0]) % 128 == 0
    ctxl = int(ctx_len[0])

    # rope tables [64, N] indexed (freq, token)
    inv = _llama31_inv_freq(D)
    ang = pos_ids.astype(np.float32)[:, None] * inv[None, :]
    cosT = np.ascontiguousarray(np.cos(ang).T).astype(BF16NP)
    sinT = np.ascontiguousarray(np.sin(ang).T).astype(BF16NP)

    # gather paged KV context (positions 0..ctxl-1 per sequence)
    cpos = np.arange(ctxl)
    pages = kpi[kpp[:-1][:, None] + (cpos[None, :] // page)]  # [B, ctxl]
    slots = np.broadcast_to(cpos % page, (b_sz, ctxl))
    Kc = kvc[pages, 0, slots]  # [B, ctxl, HKV, D]
    Vc = kvc[pages, 1, slots]

    # per-chunk causal mask for the new-kv block, [128, 4*512]:
    # chunk jn holds rows kv_rel in [jn*128,(jn+1)*128) vs all 512 q_rel cols
    qr = np.arange(qo_len)
    mbig = np.where(qr[:, None] <= qr[None, :], 0.0, -1e30).astype(np.float32)
    msk = np.ascontiguousarray(
        np.concatenate(
            [mbig[i * 128 : (i + 1) * 128] for i in range(qo_len // 128)], axis=1
        )
    ).astype(BF16NP)
    hT = np.ascontiguousarray(hs.T).astype(BF16NP)  # [HID, N] (full, all cores)

    Wq4 = Wq.reshape(HQ, D, HID)
    Wk4 = Wk.reshape(HKV, D, HID)
    Wv4 = Wv.reshape(HKV, D, HID)

    in_maps = []
    for i in range(NCORES):
        wqT = np.ascontiguousarray(
            Wq4[i * HQL : (i + 1) * HQL].reshape(HQL * D, HID).T
        ).astype(BF16NP)
        wkT = np.ascontiguousarray(Wk4[i].T).astype(BF16NP)
        wvT = np.ascontiguousarray(Wv4[i].T).astype(BF16NP)
        # column-parallel o_proj slice: out cols [i*512,(i+1)*512)
        woTc = np.ascontiguousarray(Wo[i * QO : (i + 1) * QO, :].T).astype(BF16NP)
        kctxT = np.ascontiguousarray(
            Kc[:, :, i, :].reshape(b_sz * ctxl, D).T
        ).astype(FP8NP)
        vctx = np.ascontiguousarray(
            Vc[:, :, i, :].reshape(-1, 128, D).transpose(1, 0, 2).reshape(128, b_sz * ctxl)
        ).astype(FP8NP)
        in_maps.append(
            dict(hT=hT, wqT=wqT, wkT=wkT, wvT=wvT, woTc=woTc, kctxT=kctxT,
                 vctx=vctx, cosT=cosT, sinT=sinT, msk=msk)
        )
    return in_maps, ctxl


def build_program(ctxl):
    KVL = ctxl + QO  # kv length per sequence (2048)
    CC = ctxl // 128  # context chunks per sequence (12)
    KC = KVL // 128  # total kv chunks per sequence (16)

    nc = bacc.Bacc("TRN2", debug=False, num_devices=NCORES)
    hT = nc.dram_tensor("hT", [HID, N], BF16, kind="ExternalInput").ap()
    wqT = nc.dram_tensor("wqT", [HID, HQL * D], BF16, kind="ExternalInput").ap()
    wkT = nc.dram_tensor("wkT", [HID, D], BF16, kind="ExternalInput").ap()
    wvT = nc.dram_tensor("wvT", [HID, D], BF16, kind="ExternalInput").ap()
    woTc = nc.dram_tensor("woTc", [HQ * D, QO], BF16, kind="ExternalInput").ap()
    kctxT = nc.dram_tensor("kctxT", [D, B * ctxl], FP8, kind="ExternalInput").ap()
    vctx = nc.dram_tensor("vctx", [P, B * ctxl], FP8, kind="ExternalInput").ap()
    cosT = nc.dram_tensor("cosT", [D // 2, N], BF16, kind="ExternalInput").ap()
    sinT = nc.dram_tensor("sinT", [D // 2, N], BF16, kind="ExternalInput").ap()
    msk = nc.dram_tensor("msk", [P, (QO // 128) * QO], BF16, kind="ExternalInput").ap()
    outT = nc.dram_tensor("outT", [QO, N], BF16, kind="ExternalOutput").ap()
    ones_c = nc.inline_tensor(np.ones((P, P), BF16NP), name="ones_c").ap()

    rg = [list(range(NCORES))]

    with tile.TileContext(nc) as tc:
        with tc.tile_pool(name="dram", bufs=1, space="DRAM") as dram:
            # tiny warm-up AllGather to absorb the first-collective barrier
            # while projections run
            wu_in = dram.tile([P, 8], BF16, name="wu_in")
            wu_out = dram.tile([NCORES * P, 8], BF16, addr_space="Shared", name="wu_out")
            nc.gpsimd.collective_compute(
                "AllGather", ALU.bypass, replica_groups=rg,
                ins=[wu_in[:]], outs=[wu_out[:]])
            # per-batch O all-gather buffers
            oins = [dram.tile([HQL * D, QO], BF16, tag=f"oin{b}", name=f"oin{b}")
                    for b in range(B)]
            oouts = [dram.tile([HQ * D, QO], BF16, addr_space="Shared",
                               tag=f"oout{b}", name=f"oout{b}") for b in range(B)]

            with tc.tile_pool(name="resident", bufs=1) as res:
                # q_sb: batch-major: col = b*2048 + m*512 + q
                q_sb = res.tile([P, HQL * N], BF16)
                kn_sb = res.tile([P, N], BF16)   # new K^T: [d, global token]
                vn_sb = res.tile([P, N], BF16)   # new V: 128-block t at cols t*128
                cos_sb = res.tile([P, N], BF16)
                sin_sb = res.tile([P, N], BF16)
                kctx_sb = res.tile([P, B * ctxl], BF16)
                vctx_sb = res.tile([P, B * ctxl], BF16)
                msk_sb = res.tile([P, (QO // 128) * QO], BF16)
                ones_sb = res.tile([P, P], BF16)
                ident = res.tile([P, P], BF16)
                make_identity(nc, ident[:])

                # ============ Phase A: QKV projections + rope ============
                with tc.tile_pool(name="wsb", bufs=1) as wpool, \
                     tc.tile_pool(name="hstream", bufs=36) as hpool, \
                     tc.tile_pool(name="evtmp", bufs=2) as epool, \
                     tc.tile_pool(name="apsum", bufs=6, space="PSUM") as apool, \
                     tc.tile_pool(name="tpsum", bufs=2, space="PSUM") as tpool:
                    wq_sb = wpool.tile([P, KH * HQL * D], BF16)  # (kt,m) at kt*512+m*128
                    wk_sb = wpool.tile([P, KH * D], BF16)
                    wv_sb = wpool.tile([P, KH * D], BF16)
                    # first-wave DMAs, ordered so that MM(kt) inputs land ASAP:
                    # wq on sync, wk/wv on scalar, h-half0 on gpsimd (below)
                    for kt in range(KH):
                        nc.sync.dma_start(wq_sb[:, kt * 512:(kt + 1) * 512],
                                          wqT[kt * 128:(kt + 1) * 128, :])
                        nc.scalar.dma_start(wk_sb[:, kt * 128:(kt + 1) * 128],
                                            wkT[kt * 128:(kt + 1) * 128, :])
                        nc.scalar.dma_start(wv_sb[:, kt * 128:(kt + 1) * 128],
                                            wvT[kt * 128:(kt + 1) * 128, :])
                    # needed from first rope evict (~30us in)
                    nc.scalar.dma_start(cos_sb[0:64, :], cosT)
                    nc.scalar.dma_start(cos_sb[64:128, :], cosT)
                    nc.scalar.dma_start(sin_sb[0:64, :], sinT)
                    nc.scalar.dma_start(sin_sb[64:128, :], sinT)
                    nc.sync.dma_start(ones_sb[:], ones_c)


                    def wslice(m, kt):
                        if m < HQL:
                            return wq_sb[:, kt * 512 + m * 128: kt * 512 + (m + 1) * 128]
                        if m == HQL:
                            return wk_sb[:, kt * 128:(kt + 1) * 128]
                        return wv_sb[:, kt * 128:(kt + 1) * 128]

                    def rope_evict(dst_ap, src_sb, qoff):
                        """rope from f32 SBUF tile [128,512] -> dst bf16 [128,512]."""
                        cs = cos_sb[:, qoff:qoff + 512]
                        sn = sin_sb[:, qoff:qoff + 512]
                        t1 = epool.tile([64, 512], F32, tag="t1")
                        t2 = epool.tile([64, 512], F32, tag="t2")
                        t3 = epool.tile([64, 512], F32, tag="t3")
                        t4 = epool.tile([64, 512], F32, tag="t4")
                        nc.vector.tensor_tensor(t1[:], src_sb[0:64, :], cs[0:64, :], ALU.mult)
                        nc.vector.tensor_tensor(t2[:], src_sb[64:128, :], sn[64:128, :], ALU.mult)
                        nc.vector.tensor_tensor(dst_ap[0:64, :], t1[:], t2[:], ALU.subtract)
                        nc.gpsimd.tensor_tensor(t3[:], src_sb[64:128, :], cs[64:128, :], ALU.mult)
                        nc.gpsimd.tensor_tensor(t4[:], src_sb[0:64, :], sn[0:64, :], ALU.mult)
                        nc.gpsimd.tensor_tensor(dst_ap[64:128, :], t3[:], t4[:], ALU.add)

                    # h tiles per (half, kt): [128, 1024]
                    def evict(m, quarter, acc):
                        b, qoff = quarter, quarter * 512
                        if m < 5:
                            asb = epool.tile([P, 512], F32, tag="asb")
                            nc.scalar.activation(asb[:], acc[:], AF.Copy)
                            if m < HQL:
                                dst = q_sb[:, b * (HQL * QO) + m * QO:
                                           b * (HQL * QO) + (m + 1) * QO]
                            else:
                                dst = kn_sb[:, qoff:qoff + 512]
                            rope_evict(dst, asb, qoff)
                        else:
                            vt = epool.tile([P, 512], BF16, tag="vt")
                            nc.scalar.activation(vt[:], acc[:], AF.Copy)
                            tp = tpool.tile([P, 4, P], BF16, tag="tp", name=f"tp{quarter}")
                            for t in range(4):
                                nc.tensor.transpose(
                                    tp[:, t, :], vt[:, t * 128:(t + 1) * 128],
                                    ident[:])
                            nc.vector.tensor_copy(
                                vn_sb[:, qoff:qoff + 512], tp[:, :, :])

                    # v (m=5) and k (m=4) first so the PE transposes and
                    # vector copies clear PSUM well before the phase ends
                    MORD = [5, 4, 0, 1, 2, 3]
                    hts = {}
                    for half in range(2):
                        hq_dma = nc.gpsimd if half == 0 else nc.sync
                        for kt in range(KH):
                            t = hpool.tile([P, 1024], BF16, tag="h", name=f"h{half}_{kt}")
                            hq_dma.dma_start(
                                t[:], hT[kt * 128:(kt + 1) * 128,
                                         half * 1024:(half + 1) * 1024])
                            hts[(half, kt)] = t
                        if half == 0:
                            # ctx tensors: needed only from attention onward
                            nc.gpsimd.dma_start(kctx_sb[:], kctxT)
                            nc.gpsimd.dma_start(vctx_sb[:], vctx)
                            nc.sync.dma_start(msk_sb[:], msk)
                        for qq in range(2):
                            quarter = half * 2 + qq
                            if quarter == 0:
                                # m-inner: consume each (w,h) kt-tile for all
                                # 6 outputs as soon as its DMA lands
                                accs = [apool.tile([P, 512], F32, tag="acc",
                                                   name=f"acc0_{m}") for m in range(6)]
                                for kt in range(KH):
                                    for m in MORD:
                                        nc.tensor.matmul(
                                            accs[m][:], wslice(m, kt),
                                            hts[(half, kt)][:, qq * 512:(qq + 1) * 512],
                                            start=(kt == 0), stop=(kt == KH - 1))
                                for m in MORD:
                                    evict(m, quarter, accs[m])
                            else:
                                for m in MORD:
                                    acc = apool.tile([P, 512], F32, tag="acc",
                                                     name=f"acc{quarter}_{m}")
                                    for kt in range(KH):
                                        nc.tensor.matmul(
                                            acc[:], wslice(m, kt),
                                            hts[(half, kt)][:, qq * 512:(qq + 1) * 512],
                                            start=(kt == 0), stop=(kt == KH - 1))
                                    evict(m, quarter, acc)

                # ============ Phase B: attention ============
                with tc.tile_pool(name="wosb", bufs=1) as wopool, \
                     tc.tile_pool(name="ovstream", bufs=32) as ovpool:
                    woc_sb = wopool.tile([P, KH * QO], BF16)  # (kt,ob) at kt*512+ob*128
                    for kt in range(KH):
                        nc.sync.dma_start(woc_sb[:, kt * 512:(kt + 1) * 512],
                                          woTc[kt * 128:(kt + 1) * 128, :])
                    ovs = {}

                    def prefetch_ovs(bp):
                        for kt in range(KH):
                            ov = ovpool.tile([P, 1024], BF16, tag="ov", name=f"ov{bp}_{kt}")
                            dq = (nc.sync, nc.gpsimd)[kt % 2]
                            dq.dma_start(ov[:, 0:512],
                                         oouts[2 * bp][kt * 128:(kt + 1) * 128, :])
                            dq.dma_start(ov[:, 512:1024],
                                         oouts[2 * bp + 1][kt * 128:(kt + 1) * 128, :])
                            ovs[(bp, kt)] = ov

                    with tc.tile_pool(name="spsum", bufs=2, space="PSUM") as spool, \
                         tc.tile_pool(name="opsum", bufs=1, space="PSUM") as opool, \
                         tc.tile_pool(name="dpsum", bufs=1, space="PSUM") as dpool, \
                         tc.tile_pool(name="ptile", bufs=4) as ppool, \
                         tc.tile_pool(name="rtile", bufs=2) as rpool, \
                         tc.tile_pool(name="osb", bufs=4) as osbpool:
                        LAG = 2  # PV/ones trail scores/exp by 2 chunks

                        for b in range(B):
                            for hg in range(2):
                                po = opool.tile([P, 1024], F32, tag="po", name=f"po{b}_{hg}")
                                pd = dpool.tile([P, 1024], F32, tag="pd", name=f"pd{b}_{hg}")
                                qbase = b * (HQL * QO) + hg * 1024
                                pts = [None] * KC

                                def pv_ones(c):
                                    if c < CC:
                                        vsl = vctx_sb[:, (b * CC + c) * 128:
                                                      (b * CC + c + 1) * 128]
                                    else:
                                        jn = c - CC
                                        vsl = vn_sb[:, (b * 4 + jn) * 128:
                                                    (b * 4 + jn + 1) * 128]
                                    st_, sp_ = (c == 0), (c == KC - 1)
                                    for i in range(2):
                                        nc.tensor.matmul(
                                            po[:, i * 512:(i + 1) * 512], vsl,
                                            pts[c][:, i * 512:(i + 1) * 512],
                                            start=st_, stop=sp_)
                                    for i in range(2):
                                        nc.tensor.matmul(
                                            pd[:, i * 512:(i + 1) * 512], ones_sb[:],
                                            pts[c][:, i * 512:(i + 1) * 512],
                                            start=st_, stop=sp_)

                                for c in range(KC):
                                    if c < CC:
                                        kl = kctx_sb[:, b * ctxl + c * 128:
                                                     b * ctxl + (c + 1) * 128]
                                    else:
                                        jn = c - CC
                                        kl = kn_sb[:, b * QO + jn * 128:
                                                   b * QO + (jn + 1) * 128]
                                    st = spool.tile([P, 1024], F32, tag="st", name=f"st{c%2}")
                                    for i in range(2):
                                        nc.tensor.matmul(
                                            st[:, i * 512:(i + 1) * 512], kl,
                                            q_sb[:, qbase + i * 512: qbase + (i + 1) * 512],
                                            start=True, stop=True)
                                    if c >= CC:
                                        jn = c - CC
                                        for i in range(2):
                                            nc.vector.tensor_tensor(
                                                st[:, i * 512:(i + 1) * 512],
                                                st[:, i * 512:(i + 1) * 512],
                                                msk_sb[:, jn * QO:(jn + 1) * QO],
                                                ALU.add)
                                    pt = ppool.tile([P, 1024], BF16, tag="pt", name=f"pt{c%4}")
                                    nc.scalar.activation(pt[:], st[:], AF.Exp,
                                                         scale=SM_SCALE)
                                    pts[c] = pt
                                    if c >= LAG:
                                        pv_ones(c - LAG)
                                for c in range(KC - LAG, KC):
                                    pv_ones(c)
                                rsb = rpool.tile([P, 1024], F32, tag="rsb")
                                nc.vector.reciprocal_approx_fast(rsb[:], pd[:])
                                ot = osbpool.tile([P, 1024], BF16)
                                nc.vector.tensor_tensor(ot[:], po[:], rsb[:], ALU.mult)
                                for i in range(2):
                                    m = hg * 2 + i
                                    nc.sync.dma_start(
                                        oins[b][m * 128:(m + 1) * 128, :],
                                        ot[:, i * 512:(i + 1) * 512])
                            nc.gpsimd.collective_compute(
                                "AllGather", ALU.bypass, replica_groups=rg,
                                ins=[oins[b][:]], outs=[oouts[b][:]])
                            # prefetch AG results for o_proj as soon as ready
                            if b == 1:
                                prefetch_ovs(0)

                    # ============ Phase C: column-parallel o_proj ============
                    with tc.tile_pool(name="cpsum", bufs=4, space="PSUM") as cpool, \
                         tc.tile_pool(name="outsb", bufs=3) as outpool:
                        for bp in range(2):
                            pcs = [cpool.tile([P, 1024], F32, tag="pc", name=f"pc{bp}_{ob}")
                                   for ob in range(QO // 128)]
                            for kt in range(KH):
                                for ob in range(QO // 128):
                                    for i in range(2):
                                        nc.tensor.matmul(
                                            pcs[ob][:, i * 512:(i + 1) * 512],
                                            woc_sb[:, kt * 512 + ob * 128:
                                                   kt * 512 + (ob + 1) * 128],
                                            ovs[(bp, kt)][:, i * 512:(i + 1) * 512],
                                            start=(kt == 0), stop=(kt == KH - 1))
                            if bp == 0:
                                prefetch_ovs(1)
                            for ob in range(QO // 128):
                                ot2 = outpool.tile([P, 1024], BF16)
                                nc.vector.tensor_copy(ot2[:], pcs[ob][:])
                                for i in range(2):
                                    bb = 2 * bp + i
                                    nc.sync.dma_start(
                                        outT[ob * 128:(ob + 1) * 128,
                                             bb * QO:(bb + 1) * QO],
                                        ot2[:, i * 512:(i + 1) * 512])
    nc.compile()
    return nc


_NC_CACHE = {}


def _get_program(ctxl):
    if ctxl not in _NC_CACHE:
        _NC_CACHE[ctxl] = build_program(ctxl)
    return _NC_CACHE[ctxl]


def run(inputs, trace=False):
    in_maps, ctxl = host_prep(inputs)
    nc = _get_program(ctxl)
    kw = dict(tmpdir="/tmp/trace_out") if trace else {}
    res = run_bass_kernel_spmd(nc, in_maps, core_ids=list(range(NCORES)), trace=trace, **kw)
    out = np.empty((N, HID), np.float32)
    for i, r in enumerate(res.results):
        out[:, i * QO:(i + 1) * QO] = np.asarray(r["outT"]).T.astype(np.float32)
    return out, res


def kernel(**inputs) -> np.ndarray:
    out, _ = run(inputs, trace=False)
    return out


# revision 17
# speedup vs baseline: 1.3891x; 1.0427x over previous
"""Trainium2 Bass kernel for nn_L4maAttention (llama3.1-style GQA attention layer).

Sharding: heads across 8 cores (4 Q heads + 1 KV head per core).
  - full hidden_states shipped (bf16, transposed) to every core from host;
    no on-device h AllGather -> projections start immediately.
  - q/k/v projections column-parallel + rope on device. Single-pass PSUM
    accumulation over the full 4096 contraction (quarter-token accumulator
    tiles), evicted via scalar copy + rope on vector/gpsimd.
  - paged-KV context gathered on host, shipped pre-transposed per core
    (1 kv head per core, fp8 in DRAM, bf16 in SBUF via dge-cast).
  - attention per-head local in S^T layout ([kv, q]); 2-head groups
    (1024-wide tiles); softmax denominators via ones-matmul accumulated in
    PSUM; reciprocal_approx_fast; masks added on DVE for the 4 new-kv chunks.
  - o_proj COLUMN-parallel: per-batch AllGather of bf16 attention outputs,
    each core computes a disjoint 512-column slice. Output bf16 [512, 2048].

All matmul operands bf16 (fp8 beyond the kv-ctx DRAM encoding fails the
2e-2 accuracy budget; measured on host).
"""

import math
import sys

import numpy as np

sys.path.insert(0, "/opt/trn_rl_repo")

import concourse.bass as bass  # noqa: E402
import concourse.mybir as mybir  # noqa: E402
import concourse.tile as tile  # noqa: E402
from concourse import bacc  # noqa: E402
from concourse.bass_utils import run_bass_kernel_spmd  # noqa: E402
from concourse.masks import make_identity  # noqa: E402

import ml_dtypes  # noqa: E402

# ---- problem constants (hardcoded from spec) ----
B, QO, PAGE = 4, 512, 16
HID, HQ, HKV, D = 4096, 32, 8, 128
N = B * QO  # 2048
NCORES = 8
HQL = HQ // NCORES  # 4 local q heads
ROPE_THETA = 5e5
OLD_CTX, LOW_F, HIGH_F, RSCALE = 8192.0, 1.0, 4.0, 8.0
SM_SCALE = 1.0 / math.sqrt(D)

BF16NP = ml_dtypes.bfloat16
FP8NP = ml_dtypes.float8_e4m3
F32 = mybir.dt.float32
BF16 = mybir.dt.bfloat16
FP8 = mybir.dt.float8e4
AF = mybir.ActivationFunctionType
ALU = mybir.AluOpType
P = 128
KH = HID // P  # 32 contraction chunks


def _llama31_inv_freq(d):
    inv = ROPE_THETA ** (-np.arange(0, d, 2, dtype=np.float32) / d)
    wavelen = 2.0 * np.pi / inv
    low_wl, high_wl = OLD_CTX / LOW_F, OLD_CTX / HIGH_F
    smooth = (OLD_CTX / wavelen - LOW_F) / (HIGH_F - LOW_F)
    mid = (1.0 - smooth) * inv / RSCALE + smooth * inv
    return np.where(
        wavelen > low_wl, inv / RSCALE, np.where(wavelen < high_wl, inv, mid)
    ).astype(np.float32)


def host_prep(inputs):
    """Shard + pre-transpose inputs for the 8 cores. Returns (in_maps, ctxl)."""
    hs = np.asarray(inputs["hidden_states"], np.float32)
    pos_ids = np.asarray(inputs["position_ids"], np.int32)
    kvc = np.asarray(inputs["kv_cache"], np.float32)
    kpi = np.asarray(inputs["kv_page_indices"], np.int32)
    kpp = np.asarray(inputs["kv_page_indptr"], np.int32)
    klp = np.asarray(inputs["kv_last_page_lens"], np.int32)
    qop = np.asarray(inputs["qo_indptr"], np.int32)
    Wq = np.asarray(inputs["Wq"], np.float32)
    Wk = np.asarray(inputs["Wk"], np.float32)
    Wv = np.asarray(inputs["Wv"], np.float32)
    Wo = np.asarray(inputs["Wo"], np.float32)

    n, hid = hs.shape
    b_sz = qop.shape[0] - 1
    qo_len = n // b_sz
    page = kvc.shape[2]
    pps = kpi.shape[0] // b_sz
    seq_len = (pps - 1) * page + klp  # [B]
    ctx_len = seq_len - qo_len
    assert n == N and hid == HID and b_sz == B and qo_len == QO
    assert np.all(ctx_len == ctx_len[0]) and int(ctx_len[0]) % 128 == 0
    ctxl = int(ctx_len[0])

    # rope tables [64, N] indexed (freq, token)
    inv = _llama31_inv_freq(D)
    ang = pos_ids.astype(np.float32)[:, None] * inv[None, :]
    cosT = np.ascontiguousarray(np.cos(ang).T).astype(BF16NP)
    sinT = np.ascontiguousarray(np.sin(ang).T).astype(BF16NP)

    # gather paged KV context (positions 0..ctxl-1 per sequence)
    cpos = np.arange(ctxl)
    pages = kpi[kpp[:-1][:, None] + (cpos[None, :] // page)]  # [B, ctxl]
    slots = np.broadcast_to(cpos % page, (b_sz, ctxl))
    Kc = kvc[pages, 0, slots]  # [B, ctxl, HKV, D]
    Vc = kvc[pages, 1, slots]

    # per-chunk causal mask for the new-kv block, [128, 4*512]:
    # chunk jn holds rows kv_rel in [jn*128,(jn+1)*128) vs all 512 q_rel cols
    qr = np.arange(qo_len)
    mbig = np.where(qr[:, None] <= qr[None, :], 0.0, -1e30).astype(np.float32)
    msk = np.ascontiguousarray(
        np.concatenate(
            [mbig[i * 128 : (i + 1) * 128] for i in range(qo_len // 128)], axis=1
        )
    ).astype(BF16NP)
    hT = np.ascontiguousarray(hs.T).astype(BF16NP)  # [HID, N] (full, all cores)

    Wq4 = Wq.reshape(HQ, D, HID)
    Wk4 = Wk.reshape(HKV, D, HID)
    Wv4 = Wv.reshape(HKV, D, HID)

    in_maps = []
    for i in range(NCORES):
        wqT = np.ascontiguousarray(
            Wq4[i * HQL : (i + 1) * HQL].reshape(HQL * D, HID).T
        ).astype(BF16NP)
        wkT = np.ascontiguousarray(Wk4[i].T).astype(BF16NP)
        wvT = np.ascontiguousarray(Wv4[i].T).astype(BF16NP)
        # column-parallel o_proj slice: out cols [i*512,(i+1)*512)
        woTc = np.ascontiguousarray(Wo[i * QO : (i + 1) * QO, :].T).astype(BF16NP)
        kctxT = np.ascontiguousarray(
            Kc[:, :, i, :].reshape(b_sz * ctxl, D).T
        ).astype(FP8NP)
        vctx = np.ascontiguousarray(
            Vc[:, :, i, :].reshape(-1, 128, D).transpose(1, 0, 2).reshape(128, b_sz * ctxl)
        ).astype(FP8NP)
        in_maps.append(
            dict(hT=hT, wqT=wqT, wkT=wkT, wvT=wvT, woTc=woTc, kctxT=kctxT,
                 vctx=vctx, cosT=cosT, sinT=sinT, msk=msk)
        )
    return in_maps, ctxl


def build_program(ctxl):
    KVL = ctxl + QO  # kv length per sequence (2048)
    CC = ctxl // 128  # context chunks per sequence (12)
    KC = KVL // 128  # total kv chunks per sequence (16)

    nc = bacc.Bacc("TRN2", debug=False, num_devices=NCORES)
    hT = nc.dram_tensor("hT", [HID, N], BF16, kind="ExternalInput").ap()
    wqT = nc.dram_tensor("wqT", [HID, HQL * D], BF16, kind="ExternalInput").ap()
    wkT = nc.dram_tensor("wkT", [HID, D], BF16, kind="ExternalInput").ap()
    wvT = nc.dram_tensor("wvT", [HID, D], BF16, kind="ExternalInput").ap()
    woTc = nc.dram_tensor("woTc", [HQ * D, QO], BF16, kind="ExternalInput").ap()
    kctxT = nc.dram_tensor("kctxT", [D, B * ctxl], FP8, kind="ExternalInput").ap()
    vctx = nc.dram_tensor("vctx", [P, B * ctxl], FP8, kind="ExternalInput").ap()
    cosT = nc.dram_tensor("cosT", [D // 2, N], BF16, kind="ExternalInput").ap()
    sinT = nc.dram_tensor("sinT", [D // 2, N], BF16, kind="ExternalInput").ap()
    msk = nc.dram_tensor("msk", [P, (QO // 128) * QO], BF16, kind="ExternalInput").ap()
    outT = nc.dram_tensor("outT", [QO, N], BF16, kind="ExternalOutput").ap()
    ones_c = nc.inline_tensor(np.ones((P, P), BF16NP), name="ones_c").ap()

    rg = [list(range(NCORES))]

    with tile.TileContext(nc) as tc:
        with tc.tile_pool(name="dram", bufs=1, space="DRAM") as dram:
            # tiny warm-up AllGather to absorb the first-collective barrier
            # while projections run
            wu_in = dram.tile([P, 8], BF16, name="wu_in")
            wu_out = dram.tile([NCORES * P, 8], BF16, addr_space="Shared", name="wu_out")
            nc.gpsimd.collective_compute(
                "AllGather", ALU.bypass, replica_groups=rg,
                ins=[wu_in[:]], outs=[wu_out[:]])
            # per-batch O all-gather buffers
            oins = [dram.tile([HQL * D, QO], BF16, tag=f"oin{b}", name=f"oin{b}")
                    for b in range(B)]
            oouts = [dram.tile([HQ * D, QO], BF16, addr_space="Shared",
                               tag=f"oout{b}", name=f"oout{b}") for b in range(B)]

            with tc.tile_pool(name="resident", bufs=1) as res:
                # q_sb: batch-major: col = b*2048 + m*512 + q
                q_sb = res.tile([P, HQL * N], BF16)
                kn_sb = res.tile([P, N], BF16)   # new K^T: [d, global token]
                vn_sb = res.tile([P, N], BF16)   # new V: 128-block t at cols t*128
                cos_sb = res.tile([P, N], BF16)
                sin_sb = res.tile([P, N], BF16)
                kctx_sb = res.tile([P, B * ctxl], BF16)
                vctx_sb = res.tile([P, B * ctxl], BF16)
                msk_sb = res.tile([P, (QO // 128) * QO], BF16)
                ones_sb = res.tile([P, P], BF16)
                ident = res.tile([P, P], BF16)
                make_identity(nc, ident[:])

                # ============ Phase A: QKV projections + rope ============
                with tc.tile_pool(name="wsb", bufs=1) as wpool, \
                     tc.tile_pool(name="hstream", bufs=36) as hpool, \
                     tc.tile_pool(name="evtmp", bufs=2) as epool, \
                     tc.tile_pool(name="apsum", bufs=6, space="PSUM") as apool, \
                     tc.tile_pool(name="tpsum", bufs=2, space="PSUM") as tpool:
                    wq_sb = wpool.tile([P, KH * HQL * D], BF16)  # (kt,m) at kt*512+m*128
                    wk_sb = wpool.tile([P, KH * D], BF16)
                    wv_sb = wpool.tile([P, KH * D], BF16)
                    # first-wave DMAs, ordered so that MM(kt) inputs land ASAP:
                    # wq on sync, wk/wv on scalar, h-half0 on gpsimd (below)
                    for kt in range(KH):
                        nc.sync.dma_start(wq_sb[:, kt * 512:(kt + 1) * 512],
                                          wqT[kt * 128:(kt + 1) * 128, :])
                        nc.scalar.dma_start(wk_sb[:, kt * 128:(kt + 1) * 128],
                                            wkT[kt * 128:(kt + 1) * 128, :])
                        nc.scalar.dma_start(wv_sb[:, kt * 128:(kt + 1) * 128],
                                            wvT[kt * 128:(kt + 1) * 128, :])
                    # needed from first rope evict (~30us in)
                    nc.scalar.dma_start(cos_sb[0:64, :], cosT)
                    nc.scalar.dma_start(cos_sb[64:128, :], cosT)
                    nc.scalar.dma_start(sin_sb[0:64, :], sinT)
                    nc.scalar.dma_start(sin_sb[64:128, :], sinT)
                    nc.sync.dma_start(ones_sb[:], ones_c)


                    def wslice(m, kt):
                        if m < HQL:
                            return wq_sb[:, kt * 512 + m * 128: kt * 512 + (m + 1) * 128]
                        if m == HQL:
                            return wk_sb[:, kt * 128:(kt + 1) * 128]
                        return wv_sb[:, kt * 128:(kt + 1) * 128]

                    def rope_evict(dst_ap, src_sb, qoff):
                        """rope from f32 SBUF tile [128,512] -> dst bf16 [128,512]."""
                        cs = cos_sb[:, qoff:qoff + 512]
                        sn = sin_sb[:, qoff:qoff + 512]
                        t1 = epool.tile([64, 512], F32, tag="t1")
                        t2 = epool.tile([64, 512], F32, tag="t2")
                        t3 = epool.tile([64, 512], F32, tag="t3")
                        t4 = epool.tile([64, 512], F32, tag="t4")
                        nc.vector.tensor_tensor(t1[:], src_sb[0:64, :], cs[0:64, :], ALU.mult)
                        nc.vector.tensor_tensor(t2[:], src_sb[64:128, :], sn[64:128, :], ALU.mult)
                        nc.vector.tensor_tensor(dst_ap[0:64, :], t1[:], t2[:], ALU.subtract)
                        nc.gpsimd.tensor_tensor(t3[:], src_sb[64:128, :], cs[64:128, :], ALU.mult)
                        nc.gpsimd.tensor_tensor(t4[:], src_sb[0:64, :], sn[0:64, :], ALU.mult)
                        nc.gpsimd.tensor_tensor(dst_ap[64:128, :], t3[:], t4[:], ALU.add)

                    # h tiles per (half, kt): [128, 1024]
                    def evict(m, quarter, acc):
                        b, qoff = quarter, quarter * 512
                        if m < 5:
                            asb = epool.tile([P, 512], F32, tag="asb")
                            nc.scalar.activation(asb[:], acc[:], AF.Copy)
                            if m < HQL:
                                dst = q_sb[:, b * (HQL * QO) + m * QO:
                                           b * (HQL * QO) + (m + 1) * QO]
                            else:
                                dst = kn_sb[:, qoff:qoff + 512]
                            rope_evict(dst, asb, qoff)
                        else:
                            vt = epool.tile([P, 512], BF16, tag="vt")
                            nc.scalar.activation(vt[:], acc[:], AF.Copy)
                            tp = tpool.tile([P, 4, P], BF16, tag="tp", name=f"tp{quarter}")
                            for t in range(4):
                                nc.tensor.transpose(
                                    tp[:, t, :], vt[:, t * 128:(t + 1) * 128],
                                    ident[:])
                            nc.vector.tensor_copy(
                                vn_sb[:, qoff:qoff + 512], tp[:, :, :])

                    # v (m=5) and k (m=4) first so the PE transposes and
                    # vector copies clear PSUM well before the phase ends
                    MORD = [5, 4, 0, 1, 2, 3]
                    hts = {}
                    for half in range(2):
                        hq_dma = nc.gpsimd if half == 0 else nc.sync
                        for kt in range(KH):
                            t = hpool.tile([P, 1024], BF16, tag="h", name=f"h{half}_{kt}")
                            hq_dma.dma_start(
                                t[:], hT[kt * 128:(kt + 1) * 128,
                                         half * 1024:(half + 1) * 1024])
                            hts[(half, kt)] = t
                        if half == 0:
                            # ctx tensors: needed only from attention onward
                            nc.gpsimd.dma_start(kctx_sb[:], kctxT)
                            nc.gpsimd.dma_start(vctx_sb[:], vctx)
                            nc.sync.dma_start(msk_sb[:], msk)
                        for qq in range(2):
                            quarter = half * 2 + qq
                            if quarter == 0:
                                # m-inner: consume each (w,h) kt-tile for all
                                # 6 outputs as soon as its DMA lands
                                accs = [apool.tile([P, 512], F32, tag="acc",
                                                   name=f"acc0_{m}") for m in range(6)]
                                for kt in range(KH):
                                    for m in MORD:
                                        nc.tensor.matmul(
                                            accs[m][:], wslice(m, kt),
                                            hts[(half, kt)][:, qq * 512:(qq + 1) * 512],
                                            start=(kt == 0), stop=(kt == KH - 1))
                                for m in MORD:
                                    evict(m, quarter, accs[m])
                            else:
                                for m in MORD:
                                    acc = apool.tile([P, 512], F32, tag="acc",
                                                     name=f"acc{quarter}_{m}")
                                    for kt in range(KH):
                                        nc.tensor.matmul(
                                            acc[:], wslice(m, kt),
                                            hts[(half, kt)][:, qq * 512:(qq + 1) * 512],
                                            start=(kt == 0), stop=(kt == KH - 1))
                                    evict(m, quarter, acc)

                # ============ Phase B: attention ============
                with tc.tile_pool(name="wosb", bufs=1) as wopool, \
                     tc.tile_pool(name="ovstream", bufs=32) as ovpool:
                    woc_sb = wopool.tile([P, KH * QO], BF16)  # (kt,ob) at kt*512+ob*128
                    for kt in range(KH):
                        nc.sync.dma_start(woc_sb[:, kt * 512:(kt + 1) * 512],
                                          woTc[kt * 128:(kt + 1) * 128, :])
                    ovs = {}

                    def prefetch_ovs(bp):
                        for kt in range(KH):
                            ov = ovpool.tile([P, 1024], BF16, tag="ov", name=f"ov{bp}_{kt}")
                            dq = (nc.sync, nc.gpsimd)[kt % 2]
                            dq.dma_start(ov[:, 0:512],
                                         oouts[2 * bp][kt * 128:(kt + 1) * 128, :])
                            dq.dma_start(ov[:, 512:1024],
                                         oouts[2 * bp + 1][kt * 128:(kt + 1) * 128, :])
                            ovs[(bp, kt)] = ov

                    with tc.tile_pool(name="spsum", bufs=2, space="PSUM") as spool, \
                         tc.tile_pool(name="opsum", bufs=1, space="PSUM") as opool, \
                         tc.tile_pool(name="dpsum", bufs=1, space="PSUM") as dpool, \
                         tc.tile_pool(name="ptile", bufs=6) as ppool, \
                         tc.tile_pool(name="rtile", bufs=2) as rpool, \
                         tc.tile_pool(name="osb", bufs=4) as osbpool:
                        LAG = 4  # PV/ones trail scores/exp by 4 chunks

                        for b in range(B):
                            for hg in range(2):
                                po = opool.tile([P, 1024], F32, tag="po", name=f"po{b}_{hg}")
                                pd = dpool.tile([P, 1024], F32, tag="pd", name=f"pd{b}_{hg}")
                                qbase = b * (HQL * QO) + hg * 1024
                                pts = [None] * KC

                                def pv_ones(c):
                                    # new-kv chunk jn only attends q >= jn*128
                                    off = 0 if c < CC else (c - CC) * 128
                                    if c < CC:
                                        vsl = vctx_sb[:, (b * CC + c) * 128:
                                                      (b * CC + c + 1) * 128]
                                    else:
                                        jn = c - CC
                                        vsl = vn_sb[:, (b * 4 + jn) * 128:
                                                    (b * 4 + jn + 1) * 128]
                                    st_, sp_ = (c == 0), (c == KC - 1)
                                    for i in range(2):
                                        nc.tensor.matmul(
                                            po[:, i * 512 + off:(i + 1) * 512], vsl,
                                            pts[c][:, i * 512 + off:(i + 1) * 512],
                                            start=st_, stop=sp_)
                                    for i in range(2):
                                        nc.tensor.matmul(
                                            pd[:, i * 512 + off:(i + 1) * 512], ones_sb[:],
                                            pts[c][:, i * 512 + off:(i + 1) * 512],
                                            start=st_, stop=sp_)

                                for c in range(KC):
                                    st = spool.tile([P, 1024], F32, tag="st", name=f"st{c%2}")
                                    pt = ppool.tile([P, 1024], BF16, tag="pt", name=f"pt{c%6}")
                                    if c < CC:
                                        kl = kctx_sb[:, b * ctxl + c * 128:
                                                     b * ctxl + (c + 1) * 128]
                                        for i in range(2):
                                            nc.tensor.matmul(
                                                st[:, i * 512:(i + 1) * 512], kl,
                                                q_sb[:, qbase + i * 512: qbase + (i + 1) * 512],
                                                start=True, stop=True)
                                        nc.scalar.activation(pt[:], st[:], AF.Exp,
                                                             scale=SM_SCALE)
                                    else:
                                        jn = c - CC
                                        off = jn * 128
                                        kl = kn_sb[:, b * QO + jn * 128:
                                                   b * QO + (jn + 1) * 128]
                                        for i in range(2):
                                            nc.tensor.matmul(
                                                st[:, i * 512 + off:(i + 1) * 512], kl,
                                                q_sb[:, qbase + i * 512 + off:
                                                      qbase + (i + 1) * 512],
                                                start=True, stop=True)
                                        # mask only the diagonal 128-col block
                                        for i in range(2):
                                            nc.vector.tensor_tensor(
                                                st[:, i * 512 + off: i * 512 + off + 128],
                                                st[:, i * 512 + off: i * 512 + off + 128],
                                                msk_sb[:, jn * QO + off:
                                                       jn * QO + off + 128],
                                                ALU.add)
                                        if jn == 0:
                                            nc.scalar.activation(pt[:], st[:], AF.Exp,
                                                                 scale=SM_SCALE)
                                        else:
                                            for i in range(2):
                                                nc.scalar.activation(
                                                    pt[:, i * 512 + off:(i + 1) * 512],
                                                    st[:, i * 512 + off:(i + 1) * 512],
                                                    AF.Exp, scale=SM_SCALE)
                                    pts[c] = pt
                                    if c >= LAG:
                                        pv_ones(c - LAG)
                                for c in range(KC - LAG, KC):
                                    pv_ones(c)
                                rsb = rpool.tile([P, 1024], F32, tag="rsb")
                                nc.vector.reciprocal_approx_fast(rsb[:], pd[:])
                                ot = osbpool.tile([P, 1024], BF16)
                                nc.vector.tensor_tensor(ot[:], po[:], rsb[:], ALU.mult)
                                for i in range(2):
                                    m = hg * 2 + i
                                    nc.sync.dma_start(
                                        oins[b][m * 128:(m + 1) * 128, :],
                                        ot[:, i * 512:(i + 1) * 512])
                            nc.gpsimd.collective_compute(
                                "AllGather", ALU.bypass, replica_groups=rg,
                                ins=[oins[b][:]], outs=[oouts[b][:]])
                            # prefetch AG results for o_proj as soon as ready
                            if b == 1:
                                prefetch_ovs(0)

                    # ============ Phase C: column-parallel o_proj ============
                    with tc.tile_pool(name="cpsum", bufs=4, space="PSUM") as cpool, \
                         tc.tile_pool(name="outsb", bufs=3) as outpool:
                        for bp in range(2):
                            pcs = [cpool.tile([P, 1024], F32, tag="pc", name=f"pc{bp}_{ob}")
                                   for ob in range(QO // 128)]
                            for kt in range(KH):
                                for ob in range(QO // 128):
                                    for i in range(2):
                                        nc.tensor.matmul(
                                            pcs[ob][:, i * 512:(i + 1) * 512],
                                            woc_sb[:, kt * 512 + ob * 128:
                                                   kt * 512 + (ob + 1) * 128],
                                            ovs[(bp, kt)][:, i * 512:(i + 1) * 512],
                                            start=(kt == 0), stop=(kt == KH - 1))
                            if bp == 0:
                                prefetch_ovs(1)
                            for ob in range(QO // 128):
                                ot2 = outpool.tile([P, 1024], BF16)
                                nc.vector.tensor_copy(ot2[:], pcs[ob][:])
                                for i in range(2):
                                    bb = 2 * bp + i
                                    nc.sync.dma_start(
                                        outT[ob * 128:(ob + 1) * 128,
                                             bb * QO:(bb + 1) * QO],
                                        ot2[:, i * 512:(i + 1) * 512])
    nc.compile()
    return nc


_NC_CACHE = {}


def _get_program(ctxl):
    if ctxl not in _NC_CACHE:
        _NC_CACHE[ctxl] = build_program(ctxl)
    return _NC_CACHE[ctxl]


def run(inputs, trace=False):
    in_maps, ctxl = host_prep(inputs)
    nc = _get_program(ctxl)
    kw = dict(tmpdir="/tmp/trace_out") if trace else {}
    res = run_bass_kernel_spmd(nc, in_maps, core_ids=list(range(NCORES)), trace=trace, **kw)
    out = np.empty((N, HID), np.float32)
    for i, r in enumerate(res.results):
        out[:, i * QO:(i + 1) * QO] = np.asarray(r["outT"]).T.astype(np.float32)
    return out, res


def kernel(**inputs) -> np.ndarray:
    out, _ = run(inputs, trace=False)
    return out
